# revision 5
# baseline (speedup 1.0000x reference)
"""Trainium2 Bass kernel for Llama GQA self-attention (b=2, s=2048, E=4096,
32 Q heads / 8 KV heads, RoPE, causal) sharded tensor-parallel over 8
NeuronCores (one KV-head group per core).

Per-core pipeline (identical program, per-core weight slices via inputs):
  phase 1: qkvT[768, 4096] = w_qkvT.T @ xT          (fp32r matmuls, N=256)
  phase 2: RoPE (half-layout via host-permuted weights), per (batch, head):
           scoresT[kt, qt] = kT.T @ qT (bf16, N<=512) -> exp (ScalarE, bf16)
           -> triangular mask on diagonal tiles ->
           AV: out[qt, 129] = expT.T @ [v | 1]  (denominator = ones column)
           -> normalize (per-partition reciprocal) -> transpose -> aoT fp32
  phase 3: partial_out[tok, 4096] = aoT.T @ w_outT  (fp32r, row-parallel)
Host: sum the 8 partial outputs.
"""

import os
import sys

import numpy as np

for _p in ("/opt/trn_rl_repo", "/root/.axon_site/_ro/trn_rl_repo"):
    if os.path.isdir(_p) and _p not in sys.path:
        sys.path.append(_p)

import ml_dtypes  # noqa: E402

import concourse.bass as bass  # noqa: E402
import concourse.mybir as mybir  # noqa: E402
import concourse.tile as tile  # noqa: E402
from concourse import bacc  # noqa: E402
from concourse.bass_utils import run_bass_kernel_spmd  # noqa: E402

F32 = mybir.dt.float32
F32R = mybir.dt.float32r
BF16 = mybir.dt.bfloat16
NPBF16 = ml_dtypes.bfloat16

EMBED = 4096
N_HEADS = 32
N_KV = 8
HEAD_DIM = 128
Q_PER_KV = 4
B = 2
S = 2048
TOK = B * S            # 4096
NCORES = 8
ODIM = 768             # per-core qkv rows: 4 q heads + k + v
SCALE = HEAD_DIM ** -0.5
ROPE_BASE = 10000.0

TB1 = 256              # phase-1 token block (moving N)
NB1 = TOK // TB1       # 16
ET1 = EMBED // 128     # 32 contraction tiles
NM1 = ODIM // 128      # 6 output row tiles

ALU = mybir.AluOpType
ACTF = mybir.ActivationFunctionType


def _emit(nc, tc, h):
    """Emit the whole per-core program. h: dict of DRAM APs."""
    qb = [h["qkv_d0"], h["qkv_d1"]]

    with (
        tc.tile_pool(name="consts", bufs=1) as cp,
        tc.tile_pool(name="psum", bufs=1, space="PSUM") as pp,
    ):
        tri = cp.tile([128, 128], BF16, tag="tri")
        nc.sync.dma_start(tri, h["tri"])
        idbf = cp.tile([128, 128], BF16, tag="idbf")
        nc.sync.dma_start(idbf, h["idbf"])
        id32 = cp.tile([128, 128], F32, tag="id32")
        nc.sync.dma_start(id32, h["id32"])
        cosT = cp.tile([128, S], F32, tag="cos")
        nc.sync.dma_start(cosT, h["cos"])
        sinT = cp.tile([128, S], F32, tag="sin")
        nc.sync.dma_start(sinT, h["sin"])

        # ---------------- phase 1: qkv projection ----------------
        with tc.tile_pool(name="p1", bufs=1) as p1:
            wq = p1.tile([128, ET1, ODIM], F32R, tag="wq")
            wq_src = h["wqkvT"].rearrange("(t p) m -> p t m", p=128)
            for m in range(NM1):
                nc.sync.dma_start(
                    wq[:, :, m * 128 : (m + 1) * 128],
                    wq_src[:, :, m * 128 : (m + 1) * 128],
                )
            for n in range(NB1):
                xb = p1.tile([128, ET1, TB1], F32R, tag="xb", bufs=2)
                nc.sync.dma_start(
                    xb,
                    h["xT"].rearrange("(t p) c -> p t c", p=128)[
                        :, :, n * TB1 : (n + 1) * TB1
                    ],
                )
                b = (n * TB1) // S
                col = (n * TB1) % S
                for m in range(NM1):
                    ps = pp.tile([128, TB1], F32, tag="sc", bufs=4)
                    for t in range(ET1):
                        nc.tensor.matmul(
                            ps,
                            lhsT=wq[:, t, m * 128 : (m + 1) * 128],
                            rhs=xb[:, t, :],
                            start=(t == 0),
                            stop=(t == ET1 - 1),
                        )
                    st = p1.tile([128, TB1], F32, tag="st", bufs=4)
                    nc.scalar.copy(st, ps)
                    nc.sync.dma_start(
                        qb[b][m * 128 : (m + 1) * 128, col : col + TB1], st
                    )

        # ---------------- phases 2+3 ----------------
        with tc.tile_pool(name="p2", bufs=1) as p2:
            aoT = []
            for hh in range(4):
                t_ = p2.tile([128, TOK], F32R, tag="aoT", bufs=4, name=f"aoT{hh}")
                aoT.append(t_)

            def op_chunks(tt_list):
                # out_partial[tok, :] = sum_head aoT[h].T @ w_outT[h*128:...]
                for ob in range(8):
                    wo = p2.tile([128, 4, 512], F32R, tag="wo", bufs=2)
                    nc.sync.dma_start(
                        wo,
                        h["woutT"].rearrange("(e p) o -> p e o", p=128)[
                            :, :, ob * 512 : (ob + 1) * 512
                        ],
                    )
                    for tt in tt_list:
                        ps = pp.tile([128, 512], F32, tag="sc", bufs=4)
                        for et in range(4):
                            nc.tensor.matmul(
                                ps,
                                lhsT=aoT[et][:, tt * 128 : (tt + 1) * 128],
                                rhs=wo[:, et, :],
                                start=(et == 0),
                                stop=(et == 3),
                            )
                        st = p2.tile([128, 512], F32, tag="ost", bufs=3)
                        if (tt + ob) % 2 == 0:
                            nc.vector.tensor_copy(st, ps)
                        else:
                            nc.scalar.copy(st, ps)
                        nc.sync.dma_start(
                            h["out"][tt * 128 : (tt + 1) * 128, ob * 512 : (ob + 1) * 512],
                            st,
                        )
                        yield

            opgen = None

            for b in range(B):
                # ---- v -> bf16 -> transpose -> v_aug with ones column ----
                # (emitted before RoPE: gives PE immediate work at the
                # phase-1 -> phase-2 transition while RoPE runs on DVE)
                vraw = p2.tile([128, S], F32, tag="raw", bufs=2)
                nc.sync.dma_start(vraw, qb[b][640:768, :])
                vbf = p2.tile([128, S], BF16, tag="vbf", bufs=2)
                nc.vector.tensor_copy(vbf, vraw)
                vaug = []
                for vt in range(16):
                    pst = pp.tile([128, 128], BF16, tag="tr", bufs=2)
                    nc.tensor.matmul(
                        pst,
                        lhsT=vbf[:, vt * 128 : (vt + 1) * 128],
                        rhs=idbf,
                        is_transpose=True,
                    )
                    va = p2.tile([128, 132], BF16, tag="vaug", bufs=20)
                    nc.gpsimd.memset(va[:, 128:129], 1.0)
                    nc.vector.tensor_copy(va[:, 0:128], pst)
                    vaug.append(va)

                # ---- RoPE on k first (scores gate on it), then q heads ----
                rot = [None] * 5
                for hs in (4, 0, 1, 2, 3):
                    raw = p2.tile([128, S], F32, tag="raw", bufs=2)
                    nc.sync.dma_start(raw, qb[b][hs * 128 : (hs + 1) * 128, :])
                    rt = p2.tile([128, S], F32, tag="rt", bufs=2)
                    nc.vector.tensor_scalar_mul(rt[0:64, :], raw[64:128, :], -1.0)
                    nc.vector.tensor_copy(rt[64:128, :], raw[0:64, :])
                    nc.vector.tensor_mul(raw, raw, cosT)
                    nc.vector.tensor_mul(rt, rt, sinT)
                    r_ = p2.tile([128, S], BF16, tag="rot", bufs=8, name=f"rot{b}_{hs}")
                    nc.vector.tensor_tensor(r_, raw, rt, ALU.add)
                    rot[hs] = r_

                # ---- attention per q head ----
                kr = rot[4]
                for hh in range(4):
                    qr = rot[hh]
                    for j in range(4):
                        expt = {}
                        for t in range(4 * j + 4):
                            c0 = 128 * (t - 4 * j) if t >= 4 * j else 0
                            ps = pp.tile([128, 512], F32, tag="sc", bufs=4)
                            nc.tensor.matmul(
                                ps[:, c0:512],
                                lhsT=kr[:, t * 128 : (t + 1) * 128],
                                rhs=qr[:, j * 512 + c0 : (j + 1) * 512],
                                start=True,
                                stop=True,
                            )
                            et = p2.tile([128, 512], BF16, tag="exp", bufs=17)
                            nc.scalar.activation(
                                et[:, c0:512], ps[:, c0:512], ACTF.Exp, scale=SCALE
                            )
                            if t >= 4 * j:
                                nc.vector.tensor_mul(
                                    et[:, c0 : c0 + 128], et[:, c0 : c0 + 128], tri
                                )
                            expt[t] = et
                        for u in range(4):
                            nkt = 4 * j + u + 1
                            av = pp.tile([128, 132], F32, tag="av", bufs=2)
                            for t in range(nkt):
                                nc.tensor.matmul(
                                    av[:, 0:129],
                                    lhsT=expt[t][:, u * 128 : (u + 1) * 128],
                                    rhs=vaug[t][:, 0:129],
                                    start=(t == 0),
                                    stop=(t == nkt - 1),
                                )
                            rec = p2.tile([128, 1], F32, tag="rec", bufs=2)
                            nc.vector.reciprocal(rec, av[:, 128:129])
                            ao = p2.tile([128, 128], F32, tag="ao", bufs=2)
                            nc.vector.tensor_scalar_mul(ao, av[:, 0:128], rec)
                            pst = pp.tile([128, 128], F32, tag="tr", bufs=2)
                            nc.tensor.matmul(
                                pst, lhsT=ao, rhs=id32, is_transpose=True
                            )
                            tok0 = b * S + j * 512 + u * 128
                            nc.vector.tensor_copy(
                                aoT[hh][:, tok0 : tok0 + 128], pst
                            )
                        # interleave batch-0 output-projection chunks during
                        # batch-1 attention so PE stays busy while ScalarE
                        # works through the exp backlog
                        if opgen is not None:
                            for _ in range(8):
                                next(opgen, None)

                if b == 0:
                    opgen = op_chunks(list(range(16)))

            if opgen is not None:
                for _ in opgen:
                    pass
            for _ in op_chunks(list(range(16, 32))):
                pass


def _declare(nc):
    h = {}
    h["xT"] = nc.dram_tensor("xT", [EMBED, TOK], F32R, kind="ExternalInput").ap()
    h["wqkvT"] = nc.dram_tensor("wqkvT", [EMBED, ODIM], F32R, kind="ExternalInput").ap()
    h["woutT"] = nc.dram_tensor("woutT", [512, EMBED], F32R, kind="ExternalInput").ap()
    h["cos"] = nc.dram_tensor("cosT", [128, S], F32, kind="ExternalInput").ap()
    h["sin"] = nc.dram_tensor("sinT", [128, S], F32, kind="ExternalInput").ap()
    h["tri"] = nc.dram_tensor("tri", [128, 128], BF16, kind="ExternalInput").ap()
    h["idbf"] = nc.dram_tensor("idbf", [128, 128], BF16, kind="ExternalInput").ap()
    h["id32"] = nc.dram_tensor("id32", [128, 128], F32, kind="ExternalInput").ap()
    h["qkv_d0"] = nc.dram_tensor("qkv_d0", [ODIM, S], F32).ap()
    h["qkv_d1"] = nc.dram_tensor("qkv_d1", [ODIM, S], F32).ap()
    h["out"] = nc.dram_tensor("out", [TOK, EMBED], F32, kind="ExternalOutput").ap()
    return h


_CACHE = {}


def _get_nc():
    if "nc" not in _CACHE:
        nc = bacc.Bacc(None, target_bir_lowering=False, debug=False)
        h = _declare(nc)
        with tile.TileContext(nc) as tc:
            _emit(nc, tc, h)
        nc.compile()
        _CACHE["nc"] = nc
    return _CACHE["nc"]


def _prep_in_maps(x, w_qkv, w_out):
    x = np.asarray(x, dtype=np.float32)
    w_qkv = np.asarray(w_qkv, dtype=np.float32)
    w_out = np.asarray(w_out, dtype=np.float32)

    xT = np.ascontiguousarray(x.reshape(TOK, EMBED).T)

    # RoPE tables in half-layout (rows 0..63 pair j, rows 64..127 duplicate)
    invf = ROPE_BASE ** (-np.arange(0, HEAD_DIM, 2, dtype=np.float32) / HEAD_DIM)
    ang = invf[:, None].astype(np.float64) * np.arange(S, dtype=np.float64)[None, :]
    cosT = np.concatenate([np.cos(ang), np.cos(ang)], axis=0).astype(np.float32)
    sinT = np.concatenate([np.sin(ang), np.sin(ang)], axis=0).astype(np.float32)

    tri = np.triu(np.ones((128, 128), dtype=np.float32)).astype(NPBF16)
    idbf = np.eye(128, dtype=np.float32).astype(NPBF16)
    id32 = np.eye(128, dtype=np.float32)

    # interleaved -> half-layout permutation of the head dim, applied to the
    # q/k rows of the weight (scores are invariant to a shared permutation)
    perm = np.concatenate([np.arange(0, 128, 2), np.arange(1, 128, 2)])

    in_maps = []
    for c in range(NCORES):
        ws = w_qkv[c * ODIM : (c + 1) * ODIM].copy()
        for hb in range(5):  # 4 q heads + k
            ws[hb * 128 : (hb + 1) * 128] = ws[hb * 128 : (hb + 1) * 128][perm]
        wqkvT = np.ascontiguousarray(ws.T)
        woutT = np.ascontiguousarray(w_out[:, c * 512 : (c + 1) * 512].T)
        in_maps.append(
            {
                "xT": xT,
                "wqkvT": wqkvT,
                "woutT": woutT,
                "cosT": cosT,
                "sinT": sinT,
                "tri": tri,
                "idbf": idbf,
                "id32": id32,
            }
        )
    return in_maps


def _run(inputs, trace=False):
    nc = _get_nc()
    in_maps = _prep_in_maps(inputs["x"], inputs["w_qkv"], inputs["w_out"])
    res = run_bass_kernel_spmd(nc, in_maps, list(range(NCORES)), trace=trace)
    acc = np.zeros((TOK, EMBED), dtype=np.float32)
    for r in res.results:
        acc += np.asarray(r["out"], dtype=np.float32)
    out = acc.reshape(B, S, EMBED)
    return out, res.exec_time_ns


def kernel(**inputs):
    out, _ = _run(inputs, trace=False)
    return out


# revision 7
# speedup vs baseline: 1.1057x; 1.1057x over previous
"""Trainium2 Bass kernel for Llama GQA self-attention (b=2, s=2048, E=4096,
32 Q heads / 8 KV heads, RoPE, causal) sharded tensor-parallel over 8
NeuronCores (one KV-head group per core).

Per-core pipeline (identical program, per-core weight slices via inputs):
  phase 1: qkvT[768, 4096] = w_qkvT.T @ xT          (fp32r matmuls, N=256)
  phase 2: RoPE (half-layout via host-permuted weights), per (batch, head):
           scoresT[kt, qt] = kT.T @ qT (bf16, N<=512) -> exp (ScalarE, bf16)
           -> triangular mask on diagonal tiles ->
           AV: out[qt, 129] = expT.T @ [v | 1]  (denominator = ones column)
           -> normalize (per-partition reciprocal) -> transpose -> aoT fp32
  phase 3: partial_out[tok, 4096] = aoT.T @ w_outT  (fp32r, row-parallel)
Host: sum the 8 partial outputs.
"""

import os
import sys

import numpy as np

for _p in ("/opt/trn_rl_repo", "/root/.axon_site/_ro/trn_rl_repo"):
    if os.path.isdir(_p) and _p not in sys.path:
        sys.path.append(_p)

import ml_dtypes  # noqa: E402

import concourse.bass as bass  # noqa: E402
import concourse.mybir as mybir  # noqa: E402
import concourse.tile as tile  # noqa: E402
from concourse import bacc  # noqa: E402
from concourse.bass_utils import run_bass_kernel_spmd  # noqa: E402

F32 = mybir.dt.float32
F32R = mybir.dt.float32r
BF16 = mybir.dt.bfloat16
NPBF16 = ml_dtypes.bfloat16

EMBED = 4096
N_HEADS = 32
N_KV = 8
HEAD_DIM = 128
Q_PER_KV = 4
B = 2
S = 2048
TOK = B * S            # 4096
NCORES = 8
ODIM = 768             # per-core qkv rows: 4 q heads + k + v
SCALE = HEAD_DIM ** -0.5
ROPE_BASE = 10000.0

TB1 = 256              # phase-1 token block (moving N)
NB1 = TOK // TB1       # 16
ET1 = EMBED // 128     # 32 contraction tiles
NM1 = ODIM // 128      # 6 output row tiles

ALU = mybir.AluOpType
ACTF = mybir.ActivationFunctionType


def _emit(nc, tc, h):
    """Emit the whole per-core program. h: dict of DRAM APs."""
    qb = [h["qkv_d0"], h["qkv_d1"]]

    with (
        tc.tile_pool(name="consts", bufs=1) as cp,
        tc.tile_pool(name="psum", bufs=1, space="PSUM") as pp,
    ):
        tri = cp.tile([128, 128], BF16, tag="tri")
        nc.sync.dma_start(tri, h["tri"])
        idbf = cp.tile([128, 128], BF16, tag="idbf")
        nc.sync.dma_start(idbf, h["idbf"])
        id32 = cp.tile([128, 128], F32, tag="id32")
        nc.sync.dma_start(id32, h["id32"])
        cosT = cp.tile([128, S], F32, tag="cos")
        nc.sync.dma_start(cosT, h["cos"])
        sinT = cp.tile([128, S], F32, tag="sin")
        nc.sync.dma_start(sinT, h["sin"])

        # ---------------- phase 1: qkv projection ----------------
        with tc.tile_pool(name="p1", bufs=1) as p1:
            wqm = []
            for m in range(NM1):
                w_ = p1.tile([128, ET1, 128], F32R, tag=f"wq{m}", name=f"wq{m}")
                nc.sync.dma_start(w_, h["wqkvT"][m])
                wqm.append(w_)
            for n in range(NB1):
                xb = p1.tile([128, ET1, TB1], F32R, tag="xb", bufs=2)
                nc.sync.dma_start(xb, h["xT"][n])
                b = (n * TB1) // S
                col = (n * TB1) % S
                for m in range(NM1):
                    ps = pp.tile([128, TB1], F32, tag="sc", bufs=3)
                    for t in range(ET1):
                        nc.tensor.matmul(
                            ps,
                            lhsT=wqm[m][:, t, :],
                            rhs=xb[:, t, :],
                            start=(t == 0),
                            stop=(t == ET1 - 1),
                        )
                    st = p1.tile([128, TB1], F32, tag="st", bufs=4)
                    nc.scalar.copy(st, ps)
                    nc.sync.dma_start(
                        qb[b][m * 128 : (m + 1) * 128, col : col + TB1], st
                    )

        # ---------------- phases 2+3 ----------------
        with tc.tile_pool(name="p2", bufs=1) as p2:
            aoT = []
            for hh in range(4):
                t_ = p2.tile([128, TOK], F32R, tag="aoT", bufs=4, name=f"aoT{hh}")
                aoT.append(t_)

            def op_chunks(tt_list):
                # out_partial[tok, :] = sum_head aoT[h].T @ w_outT[h*128:...]
                for ob in range(8):
                    wo = p2.tile([128, 4, 512], F32R, tag="wo", bufs=3)
                    nc.sync.dma_start(wo, h["woutT"][ob])
                    for tt in tt_list:
                        ps = pp.tile([128, 512], F32, tag="op", bufs=2)
                        for et in range(4):
                            nc.tensor.matmul(
                                ps,
                                lhsT=aoT[et][:, tt * 128 : (tt + 1) * 128],
                                rhs=wo[:, et, :],
                                start=(et == 0),
                                stop=(et == 3),
                            )
                        st = p2.tile([128, 512], F32, tag="ost", bufs=4)
                        if (tt + ob) % 2 == 0:
                            nc.vector.tensor_copy(st, ps)
                        else:
                            nc.scalar.copy(st, ps)
                        nc.sync.dma_start(
                            h["out"][tt * 128 : (tt + 1) * 128, ob * 512 : (ob + 1) * 512],
                            st,
                        )
                        yield

            opgen = None

            for b in range(B):
                # ---- v -> bf16 -> transpose -> v_aug with ones column ----
                # (emitted before RoPE: gives PE immediate work at the
                # phase-1 -> phase-2 transition while RoPE runs on DVE)
                vraw = p2.tile([128, S], F32, tag="raw", bufs=2)
                nc.sync.dma_start(vraw, qb[b][640:768, :])
                vbf = p2.tile([128, S], BF16, tag="vbf", bufs=2)
                nc.vector.tensor_copy(vbf, vraw)
                vaug = []
                for vt in range(16):
                    pst = pp.tile([128, 128], BF16, tag="tr", bufs=1)
                    nc.tensor.matmul(
                        pst,
                        lhsT=vbf[:, vt * 128 : (vt + 1) * 128],
                        rhs=idbf,
                        is_transpose=True,
                    )
                    va = p2.tile([128, 132], BF16, tag="vaug", bufs=18)
                    nc.gpsimd.memset(va[:, 128:129], 1.0)
                    nc.vector.tensor_copy(va[:, 0:128], pst)
                    vaug.append(va)

                # ---- RoPE on k first (scores gate on it), then q heads ----
                rot = [None] * 5
                for hs in (4, 0, 1, 2, 3):
                    raw = p2.tile([128, S], F32, tag="raw", bufs=2)
                    nc.sync.dma_start(raw, qb[b][hs * 128 : (hs + 1) * 128, :])
                    rt = p2.tile([128, S], F32, tag="rt", bufs=2)
                    nc.vector.tensor_scalar_mul(rt[0:64, :], raw[64:128, :], -1.0)
                    nc.vector.tensor_copy(rt[64:128, :], raw[0:64, :])
                    nc.vector.tensor_mul(raw, raw, cosT)
                    nc.vector.tensor_mul(rt, rt, sinT)
                    r_ = p2.tile([128, S], BF16, tag="rot", bufs=8, name=f"rot{b}_{hs}")
                    nc.vector.tensor_tensor(r_, raw, rt, ALU.add)
                    rot[hs] = r_

                # ---- attention per q head ----
                kr = rot[4]
                for hh in range(4):
                    qr = rot[hh]
                    for j in range(4):
                        expt = {}
                        for t in range(4 * j + 4):
                            c0 = 128 * (t - 4 * j) if t >= 4 * j else 0
                            ps = pp.tile([128, 512], F32, tag="sc", bufs=3)
                            nc.tensor.matmul(
                                ps[:, c0:512],
                                lhsT=kr[:, t * 128 : (t + 1) * 128],
                                rhs=qr[:, j * 512 + c0 : (j + 1) * 512],
                                start=True,
                                stop=True,
                            )
                            et = p2.tile([128, 512], BF16, tag="exp", bufs=16)
                            nc.scalar.activation(
                                et[:, c0:512], ps[:, c0:512], ACTF.Exp, scale=SCALE
                            )
                            if t >= 4 * j:
                                nc.vector.tensor_mul(
                                    et[:, c0 : c0 + 128], et[:, c0 : c0 + 128], tri
                                )
                            expt[t] = et
                        for u in range(4):
                            nkt = 4 * j + u + 1
                            av = pp.tile([128, 132], F32, tag="av", bufs=2)
                            for t in range(nkt):
                                nc.tensor.matmul(
                                    av[:, 0:129],
                                    lhsT=expt[t][:, u * 128 : (u + 1) * 128],
                                    rhs=vaug[t][:, 0:129],
                                    start=(t == 0),
                                    stop=(t == nkt - 1),
                                )
                            rec = p2.tile([128, 1], F32, tag="rec", bufs=2)
                            nc.vector.reciprocal(rec, av[:, 128:129])
                            ao = p2.tile([128, 128], F32, tag="ao", bufs=2)
                            nc.vector.tensor_scalar_mul(ao, av[:, 0:128], rec)
                            pst = pp.tile([128, 128], F32, tag="tr", bufs=1)
                            nc.tensor.matmul(
                                pst, lhsT=ao, rhs=id32, is_transpose=True
                            )
                            tok0 = b * S + j * 512 + u * 128
                            nc.vector.tensor_copy(
                                aoT[hh][:, tok0 : tok0 + 128], pst
                            )
                        # interleave batch-0 output-projection chunks during
                        # batch-1 attention so PE stays busy while ScalarE
                        # works through the exp backlog
                        if opgen is not None:
                            for _ in range(8):
                                next(opgen, None)

                if b == 0:
                    opgen = op_chunks(list(range(16)))

            if opgen is not None:
                for _ in opgen:
                    pass
            for _ in op_chunks(list(range(16, 32))):
                pass


def _declare(nc):
    h = {}
    h["xT"] = nc.dram_tensor("xT", [NB1, 128, ET1, TB1], F32R, kind="ExternalInput").ap()
    h["wqkvT"] = nc.dram_tensor("wqkvT", [NM1, 128, ET1, 128], F32R, kind="ExternalInput").ap()
    h["woutT"] = nc.dram_tensor("woutT", [8, 128, 4, 512], F32R, kind="ExternalInput").ap()
    h["cos"] = nc.dram_tensor("cosT", [128, S], F32, kind="ExternalInput").ap()
    h["sin"] = nc.dram_tensor("sinT", [128, S], F32, kind="ExternalInput").ap()
    h["tri"] = nc.dram_tensor("tri", [128, 128], BF16, kind="ExternalInput").ap()
    h["idbf"] = nc.dram_tensor("idbf", [128, 128], BF16, kind="ExternalInput").ap()
    h["id32"] = nc.dram_tensor("id32", [128, 128], F32, kind="ExternalInput").ap()
    h["qkv_d0"] = nc.dram_tensor("qkv_d0", [ODIM, S], F32).ap()
    h["qkv_d1"] = nc.dram_tensor("qkv_d1", [ODIM, S], F32).ap()
    h["out"] = nc.dram_tensor("out", [TOK, EMBED], F32, kind="ExternalOutput").ap()
    return h


_CACHE = {}


def _get_nc():
    if "nc" not in _CACHE:
        nc = bacc.Bacc(None, target_bir_lowering=False, debug=False)
        h = _declare(nc)
        with tile.TileContext(nc) as tc:
            _emit(nc, tc, h)
        nc.compile()
        _CACHE["nc"] = nc
    return _CACHE["nc"]


def _prep_in_maps(x, w_qkv, w_out):
    x = np.asarray(x, dtype=np.float32)
    w_qkv = np.asarray(w_qkv, dtype=np.float32)
    w_out = np.asarray(w_out, dtype=np.float32)

    xT = x.reshape(TOK, EMBED).T  # [E, TOK]
    xT = np.ascontiguousarray(
        xT.reshape(ET1, 128, NB1, TB1).transpose(2, 1, 0, 3)
    )  # [n, p, t, c] -- 32KB contiguous per (n, p)

    # RoPE tables in half-layout (rows 0..63 pair j, rows 64..127 duplicate)
    invf = ROPE_BASE ** (-np.arange(0, HEAD_DIM, 2, dtype=np.float32) / HEAD_DIM)
    ang = invf[:, None].astype(np.float64) * np.arange(S, dtype=np.float64)[None, :]
    cosT = np.concatenate([np.cos(ang), np.cos(ang)], axis=0).astype(np.float32)
    sinT = np.concatenate([np.sin(ang), np.sin(ang)], axis=0).astype(np.float32)

    tri = np.triu(np.ones((128, 128), dtype=np.float32)).astype(NPBF16)
    idbf = np.eye(128, dtype=np.float32).astype(NPBF16)
    id32 = np.eye(128, dtype=np.float32)

    # interleaved -> half-layout permutation of the head dim, applied to the
    # q/k rows of the weight (scores are invariant to a shared permutation)
    perm = np.concatenate([np.arange(0, 128, 2), np.arange(1, 128, 2)])

    in_maps = []
    for c in range(NCORES):
        ws = w_qkv[c * ODIM : (c + 1) * ODIM].copy()
        for hb in range(5):  # 4 q heads + k
            ws[hb * 128 : (hb + 1) * 128] = ws[hb * 128 : (hb + 1) * 128][perm]
        wqkvT = ws.T.reshape(ET1, 128, NM1, 128).transpose(2, 1, 0, 3)
        wqkvT = np.ascontiguousarray(wqkvT)  # [m, p, t, d]
        woutT = w_out[:, c * 512 : (c + 1) * 512].T  # [512, E]
        woutT = np.ascontiguousarray(
            woutT.reshape(4, 128, 8, 512).transpose(2, 1, 0, 3)
        )  # [ob, p, et, o]
        in_maps.append(
            {
                "xT": xT,
                "wqkvT": wqkvT,
                "woutT": woutT,
                "cosT": cosT,
                "sinT": sinT,
                "tri": tri,
                "idbf": idbf,
                "id32": id32,
            }
        )
    return in_maps


def _run(inputs, trace=False):
    nc = _get_nc()
    in_maps = _prep_in_maps(inputs["x"], inputs["w_qkv"], inputs["w_out"])
    res = run_bass_kernel_spmd(nc, in_maps, list(range(NCORES)), trace=trace)
    acc = np.zeros((TOK, EMBED), dtype=np.float32)
    for r in res.results:
        acc += np.asarray(r["out"], dtype=np.float32)
    out = acc.reshape(B, S, EMBED)
    return out, res.exec_time_ns


def kernel(**inputs):
    out, _ = _run(inputs, trace=False)
    return out
